# revision 1
# baseline (speedup 1.0000x reference)
"""Trainium2 Bass kernel for nn_BetaBinsMassAdaptive (v2: dma_gather windows).

Computes, for each batch element:
  logits = uid_w[uid] + iid_w[iid]            (gather from two [1M,5] tables)
  bins   = softmax(logits); edges = cumsum(bins)
  cdf    = betainc(alpha, beta, edges[:4]); mass = diff([0, cdf, 1])
Returns (mass, edges), both [B,5] float32.

Gather strategy (replaces 8192 per-core indirect_dma_start calls, which are
descriptor-generation bound at ~17us each):
  - tables are padded on-device to [1M, 8] bf16 (16B rows) in HBM scratch
  - one InstDMAGatherAnt per 4096 elements fetches 512B windows of 32 rows
    (idx = uid>>5 fits the gather's int16 index limit) in xbar-transpose
    layout: window u16 position w -> partition w%128, block w//128, col=elem
  - the exact row (lo = uid&31) is selected on-chip: a one-hot mask built by
    comparing a broadcast of lo against a per-partition constant, then a
    [128x8] constant matmul folds the 16 candidate partition-positions down
    to 8 output rows; uid/iid selects accumulate in PSUM -> logits [5, G]
  - logits staged to HBM [5, N] and re-read in [128, W] element-major tiles
    for the betainc pipeline (same validated GL4+series math as v1)

betainc strategy (validated to ~8e-6 max abs err vs float64):
  mass_0 = I_tau0(a,b)        + GL4 integral over [tau0, x0]
  mass_k = GL4 integral over [x_{k-1}, x_k]          (k=1,2,3)
  mass_4 = I_{1-tau4}(b,a)    + GL4 integral over [x3, tau4]
  I_tau(p,q) at FIXED tau via the 2F1 series (N=8 terms), 1/B via Stirling.

Sharding: batch split evenly across 8 NeuronCores; tables replicated.
"""
import sys

sys.path.insert(0, "/opt/trn_rl_repo")

import os as _os

import numpy as np
import ml_dtypes

_NOINC = _os.environ.get("KNOINC", "") == "1"

import concourse.bass as bass
import concourse.bacc as bacc
import concourse.mybir as mybir
from concourse.tile import TileContext
from concourse import bass_utils

F32 = mybir.dt.float32
BF16 = mybir.dt.bfloat16
I32 = mybir.dt.int32
I16 = mybir.dt.int16
AF = mybir.ActivationFunctionType
OP = mybir.AluOpType

P = 128
N_CORES = 8
B_TOTAL = 4_194_304
PER_CORE = B_TOTAL // N_CORES      # 524288
NROWS = 1_000_000
D = 5

# gather phase
G = 4096                           # elements per gather group
GW = G // 16                       # idx cols (wrap-16)
NG = PER_CORE // G                 # 128 groups
PIECE = 512                        # PE/mask piece (PSUM bank width)
NPIECE = G // PIECE                # 8
NSPLIT = 2                         # gathers per table (ring-friendly)
GS = G // NSPLIT                   # idxs per gather
NWIN = NROWS // 32                 # 31250 windows of 256 bf16

# betainc phase
CHUNK = 32768
W = CHUNK // P                     # 256
NCHUNK = PER_CORE // CHUNK         # 16
GROUPS_PER_CHUNK = CHUNK // G      # 8

TAU0 = 0.107
TAU4 = 1.0 - TAU0
NSER = 6
QT = 4
QI = 3

GL_X = {
    3: [-0.7745966692414834, 0.0, 0.7745966692414834],
    4: [-0.8611363115940526, -0.33998104358485626, 0.33998104358485626, 0.8611363115940526],
}
GL_W = {
    3: [0.5555555555555556, 0.8888888888888888, 0.5555555555555556],
    4: [0.34785484513745385, 0.6521451548625461, 0.6521451548625461, 0.34785484513745385],
}

HALF_LN_2PI = 0.9189385332046727

# table prep tiling
PREP_R = 16384                     # rows per prep tile (128 x 128 x 5)
PREP_NT = NROWS // PREP_R          # 61 full tiles
PREP_TAIL = NROWS - PREP_NT * PREP_R   # 576 = 64 x 9
PREP_STORES = PREP_NT + 1          # 62 stores per table


def _emit_table_prep(nc, pool, w_d, wp_d):
    """Pad/cast one [1M,5] f32 table to [1M,8] bf16 in HBM scratch.

    Returns the final dst tiles; a later gpsimd write to their corners (WAR
    on the store DMAs) gates the gathers behind prep completion.
    """
    v = nc.vector
    finals = []
    for t in range(PREP_NT):
        src = pool.tile([P, 128 * D], F32, name="psrc", bufs=1)
        dst = pool.tile([P, 128 * 8], BF16, name="pdst", bufs=2)
        if t >= PREP_NT - 2:
            finals.append(dst)
        sl = slice(t * PREP_R, (t + 1) * PREP_R)
        nc.sync.dma_start(src[:], w_d[sl].rearrange("(p r) d -> p (r d)", p=P))
        dstv = dst[:].rearrange("p (r d) -> p r d", d=8)
        v.memset(dstv[:, :, D:8], 0.0)
        v.tensor_copy(out=dstv[:, :, 0:D],
                      in_=src[:].rearrange("p (r d) -> p r d", d=D))
        nc.sync.dma_start(
            wp_d[sl].rearrange("(p r) d -> p (r d)", p=P), dst[:])
    # tail: 576 rows = 64 partitions x 9 rows
    src = pool.tile([64, 9 * D], F32, name="psrct", bufs=1)
    dst = pool.tile([64, 9 * 8], BF16, name="pdstt", bufs=1)
    sl = slice(PREP_NT * PREP_R, NROWS)
    nc.sync.dma_start(src[:], w_d[sl].rearrange("(p r) d -> p (r d)", p=64))
    dstv = dst[:].rearrange("p (r d) -> p r d", d=8)
    v.memset(dstv[:, :, D:8], 0.0)
    v.tensor_copy(out=dstv[:, :, 0:D],
                  in_=src[:].rearrange("p (r d) -> p r d", d=D))
    nc.sync.dma_start(
        wp_d[sl].rearrange("(p r) d -> p (r d)", p=64), dst[:])
    finals.append(dst)
    return finals


def _emit_group(nc, pool, ppool, cst, dram, g, gather_only=False):
    """Gather + select logits for elements [g*G, (g+1)*G) -> Lh columns.

    Returns the L_sb tile whose store carries this group's Lh write."""
    v = nc.vector
    s = nc.scalar
    uid_d, iid_d, uwp_d, iwp_d, Lh_d = dram
    ones16_t, a8_t, cc_t, i16_t = cst

    sl = slice(g * G, (g + 1) * G)
    win_v = {}
    for t, (idx_d, wp_d) in enumerate(((uid_d, uwp_d), (iid_d, iwp_d))):
        nm = "ui"[t]
        # All gathers on queue 0: the xbar-transpose drain path corrupts
        # data when two queues' transpose gathers run concurrently.
        qn = 0
        pb = 0
        # wrap-16 permutes only within each 256-element block: gather pos
        # i = blk*256 + r reads batch element blk*256 + (r%16)*16 + r//16,
        # so idx32[p, blk*16+w] = uid[g*G + blk*256 + p*16 + w].
        idx32 = pool.tile([16, GW], I32, name=f"idx32{nm}", bufs=2)
        nc.sync.dma_start(
            idx32[:].rearrange("p (blk w) -> p blk w", w=16),
            idx_d[sl].rearrange("(blk p w) -> p blk w", p=16, w=16))
        hi32 = pool.tile([16, GW], I32, name=f"hi32{nm}", bufs=1)
        v.tensor_scalar(out=hi32[:], in0=idx32[:], scalar1=5, scalar2=None,
                        op0=OP.arith_shift_right)
        idx16 = pool.tile([32, GW], I16, name=f"idx16{nm}", bufs=2)
        v.tensor_copy(out=idx16[pb:pb + 16, :], in_=hi32[:])
        nc.sync.dma_start(idx16[pb + 16:pb + 32, :], idx16[pb:pb + 16, :])
        lo32 = pool.tile([16, GW], I32, name=f"lo32{nm}", bufs=1)
        v.tensor_scalar(out=lo32[:], in0=idx32[:], scalar1=31, scalar2=None,
                        op0=OP.bitwise_and)
        lo_bf = pool.tile([16, GW], BF16, name=f"lobf{nm}", bufs=1)
        v.tensor_copy(out=lo_bf[:], in_=lo32[:])
        spread = pool.tile([16, GW * 16], BF16, name=f"spread{nm}", bufs=1)
        nc.gpsimd.affine_select(
            out=spread[:].rearrange("p (w j) -> p w j", j=16),
            in_=lo_bf[:].rearrange("p (w o) -> p w o", o=1).broadcast_to([16, GW, 16]),
            pattern=[[0, GW], [1, 16]],
            compare_op=OP.is_equal,
            fill=0.0,
            base=0,
            channel_multiplier=-1)
        wins = []
        for h in range(NSPLIT):
            win = pool.tile([P, 2 * GS], BF16, name=f"win{nm}{h}", bufs=2)
            # single_packet=False: coalescing would exceed the 64-desc
            # packet limit (tx descs/engine > 64) and faults the device
            nc.gpsimd.dma_gather(
                win[:].rearrange("p (b e) -> p b e", b=2),
                wp_d[:].rearrange("(w k) d -> w (k d)", k=32),
                idx16[:, h * (GS // 16):(h + 1) * (GS // 16)],
                GS, GS, 256, elem_step=256, transpose=True,
                single_packet=False, queue_num=qn)
            wins.append(win)
        win_v[nm] = (wins, spread)

    if gather_only:
        # consume win/spread trivially so Tile rotation stays sane
        zsink = pool.tile([P, 8], BF16, name="zsink", bufs=1)
        for nm in ("u", "i"):
            wins, spread = win_v[nm]
            for win in wins:
                nc.vector.tensor_copy(out=zsink[:], in_=win[:, 0:8])
        return None

    L_sb = pool.tile([8, G], BF16, name="Lsb", bufs=2)
    for sp in range(NPIECE):
        ps = slice(sp * PIECE, (sp + 1) * PIECE)
        hsp = sp // (NPIECE // NSPLIT)
        psl = slice((sp % (NPIECE // NSPLIT)) * PIECE,
                    (sp % (NPIECE // NSPLIT) + 1) * PIECE)
        gms = []
        for nm in ("u", "i"):
            wins, spread = win_v[nm]
            winv = wins[hsp][:].rearrange("p (b e) -> p b e", b=2)
            lo_ps = ppool.tile([P, PIECE], F32, name=f"lops{nm}", bufs=1,
                               space="PSUM")
            nc.tensor.matmul(out=lo_ps[:], lhsT=ones16_t[:],
                             rhs=spread[:, ps], start=True, stop=True)
            gm0 = pool.tile([P, PIECE], BF16, name=f"gm0{nm}", bufs=1)
            gm1 = pool.tile([P, PIECE], BF16, name=f"gm1{nm}", bufs=1)
            v.scalar_tensor_tensor(out=gm0[:], in0=lo_ps[:], scalar=cc_t[:, 0:1],
                                   in1=winv[:, 0, psl], op0=OP.is_equal, op1=OP.mult)
            v.scalar_tensor_tensor(out=gm1[:], in0=lo_ps[:], scalar=cc_t[:, 1:2],
                                   in1=winv[:, 1, psl], op0=OP.is_equal, op1=OP.mult)
            gms += [gm0, gm1]
        L_ps = ppool.tile([8, PIECE], F32, name="Lps", bufs=2, space="PSUM")
        for k, gm in enumerate(gms):
            nc.tensor.matmul(out=L_ps[:], lhsT=a8_t[:], rhs=gm[:],
                             start=(k == 0), stop=(k == len(gms) - 1))
        s.activation(L_sb[:, ps], L_ps[:], AF.Copy)

    nc.sync.dma_start(Lh_d[:, sl], L_sb[0:5, :])
    return L_sb


def _emit_chunk(nc, pool, dram, last_lsb, c):
    """betainc pipeline for chunk c (CHUNK elements), reading staged logits.

    last_lsb: L_sb tile of the chunk's final group. A corner write to it
    (WAR) waits for that group's Lh store; per-ring FIFO completion means
    all 8 group stores are then done, and the WAW chain onto Lt delays the
    logits load until the corner write completes."""
    v = nc.vector
    s = nc.scalar
    al_d, be_d, Lh_d, mass_d, edges_d = dram

    def t(name, width=W, dtype=F32, bufs=1):
        return pool.tile([P, width], dtype, name=name, tag=name, bufs=bufs)

    # ---- loads ----
    Lt = t("Lt", W * D, BF16, bufs=2)
    nc.sync.dma_start(last_lsb[0:1, 0:8], Lt[0:1, 0:8])      # t1: WAR on L-stores
    nc.sync.dma_start(Lt[0:1, 0:8], last_lsb[0:1, 0:8])      # t2: RAW t1 -> WAW gates load
    nc.sync.dma_start(
        Lt[:].rearrange("p (d j) -> p d j", d=D),
        Lh_d[:, c * CHUNK:(c + 1) * CHUNK].rearrange("d (q j) -> q d j", q=P))

    al = t("al", bufs=2)
    be = t("be", bufs=2)
    csl = slice(c * CHUNK, (c + 1) * CHUNK)
    nc.sync.dma_start(al[:], al_d[csl].rearrange("(p w) -> p w", p=P))
    nc.sync.dma_start(be[:], be_d[csl].rearrange("(p w) -> p w", p=P))

    # ---- softmax -> edges ----
    # E column u holds batch element c*CHUNK + q*256 + u, whose logits sit at
    # gather position j = (u%16)*16 + u//16 within the partition's Lt row.
    E = t("E", W * D)
    v.tensor_copy(
        out=E[:].rearrange("p (j16 jh d) -> p j16 jh d", jh=16, j16=16, d=D),
        in_=Lt[:].rearrange("p (d jh j16) -> p j16 jh d", d=D, jh=16, j16=16))
    s.activation(E[:], E[:], AF.Exp)
    Ev = E[:].rearrange("p (w d) -> p w d", d=D)

    cum = t("cum", W * D)
    cumv = cum[:].rearrange("p (w d) -> p w d", d=D)
    s.activation(cumv[:, :, 0], Ev[:, :, 0], AF.Copy)
    for j in range(1, D):
        v.tensor_tensor(out=cumv[:, :, j], in0=cumv[:, :, j - 1], in1=Ev[:, :, j], op=OP.add)

    invS = t("invS")
    scr = t("scr")
    v.reciprocal_approx_accurate(out=invS[:], in_=cumv[:, :, D - 1], scratch=scr[:])

    edges = t("edges", W * D, bufs=2)
    edv = edges[:].rearrange("p (w d) -> p w d", d=D)
    invSb = invS[:].rearrange("p (w o) -> p w o", o=1).broadcast_to([P, W, D])
    v.tensor_tensor(out=edv[:], in0=cumv[:], in1=invSb, op=OP.mult)

    def x(k):
        return edv[:, :, k]

    # ---- per-element prep ----
    ab = t("ab")
    v.tensor_tensor(out=ab[:], in0=al[:], in1=be[:], op=OP.add)
    A1 = t("A1")
    B1 = t("B1")
    s.activation(A1[:], al[:], AF.Identity, bias=-1.0)
    s.activation(B1[:], be[:], AF.Identity, bias=-1.0)
    inv_a = t("inv_a")
    inv_b = t("inv_b")
    v.reciprocal_approx_fast(out=inv_a[:], in_=al[:])
    v.reciprocal_approx_fast(out=inv_b[:], in_=be[:])

    # ---- lnB(a,b) via Stirling shift-2 ----
    def stirling(z, out):
        lw = t("lw")
        wt = t("wt")
        iw = t("iw")
        t1 = t("t1")
        u = t("u2")
        s.activation(lw[:], z[:], AF.Ln, bias=2.0)
        s.activation(wt[:], z[:], AF.Identity, bias=2.0)
        v.reciprocal_approx_fast(out=iw[:], in_=wt[:])
        v.scalar_tensor_tensor(out=t1[:], in0=z[:], scalar=1.5, in1=lw[:], op0=OP.add, op1=OP.mult)
        v.tensor_tensor(out=u[:], in0=iw[:], in1=iw[:], op=OP.mult)
        v.tensor_scalar(out=u[:], in0=u[:], scalar1=-1.0 / 360.0, scalar2=1.0 / 12.0, op0=OP.mult, op1=OP.add)
        v.tensor_tensor(out=u[:], in0=iw[:], in1=u[:], op=OP.mult)
        v.tensor_tensor(out=t1[:], in0=t1[:], in1=wt[:], op=OP.subtract)
        v.scalar_tensor_tensor(out=out[:], in0=u[:], scalar=HALF_LN_2PI, in1=t1[:], op0=OP.add, op1=OP.add)

    Sa = t("Sa")
    Sb = t("Sb")
    Sab = t("Sab")
    stirling(al, Sa)
    stirling(be, Sb)
    stirling(ab, Sab)
    lnB = t("lnB")
    v.tensor_tensor(out=lnB[:], in0=Sa[:], in1=Sb[:], op=OP.add)
    v.tensor_tensor(out=lnB[:], in0=lnB[:], in1=Sab[:], op=OP.subtract)
    pa = t("pa")
    pb = t("pb")
    pab = t("pab")
    v.scalar_tensor_tensor(out=pa[:], in0=al[:], scalar=1.0, in1=al[:], op0=OP.add, op1=OP.mult)
    v.scalar_tensor_tensor(out=pb[:], in0=be[:], scalar=1.0, in1=be[:], op0=OP.add, op1=OP.mult)
    v.scalar_tensor_tensor(out=pab[:], in0=ab[:], scalar=1.0, in1=ab[:], op0=OP.add, op1=OP.mult)
    s.activation(pa[:], pa[:], AF.Ln)
    s.activation(pb[:], pb[:], AF.Ln)
    s.activation(pab[:], pab[:], AF.Ln)
    v.tensor_tensor(out=lnB[:], in0=lnB[:], in1=pab[:], op=OP.add)
    v.tensor_tensor(out=pa[:], in0=pa[:], in1=pb[:], op=OP.add)
    v.tensor_tensor(out=lnB[:], in0=lnB[:], in1=pa[:], op=OP.subtract)
    iB = t("iB")
    s.activation(iB[:], lnB[:], AF.Exp, scale=-1.0)

    # ---- tail series: phi = front * 2F1(1, p+q; p+1; tau) ----
    def tail_series(p_pl, q_pl, invp, tau, out):
        lt = float(np.log(tau))
        l1t = float(np.log1p(-tau))
        e1 = t("e1")
        tt = t("tt")
        un = t("un")
        iu = t("iu")
        v.tensor_scalar(out=e1[:], in0=p_pl[:], scalar1=lt, scalar2=None, op0=OP.mult)
        v.scalar_tensor_tensor(out=e1[:], in0=q_pl[:], scalar=l1t, in1=e1[:], op0=OP.mult, op1=OP.add)
        s.activation(e1[:], e1[:], AF.Exp)
        v.tensor_tensor(out=tt[:], in0=e1[:], in1=invp[:], op=OP.mult)
        s.activation(out[:], tt[:], AF.Copy)
        for n in range(NSER):
            s.activation(un[:], p_pl[:], AF.Identity, scale=1.0 / tau, bias=(1.0 + n) / tau)
            v.reciprocal_approx_fast(out=iu[:], in_=un[:])
            v.tensor_tensor(out=tt[:], in0=tt[:], in1=iu[:], op=OP.mult)
            v.scalar_tensor_tensor(out=tt[:], in0=ab[:], scalar=float(n), in1=tt[:], op0=OP.add, op1=OP.mult)
            v.tensor_tensor(out=out[:], in0=out[:], in1=tt[:], op=OP.add)

    phi0 = t("phi0")
    phi4 = t("phi4")
    tail_series(al, be, inv_a, TAU0, phi0)
    tail_series(be, al, inv_b, 1.0 - TAU4, phi4)

    # ---- GL integrals ----
    mass = t("mass", W * D, bufs=2)
    mav = mass[:].rearrange("p (w d) -> p w d", d=D)

    dpl = t("dpl")
    hpl = t("hpl")
    mpl = t("mpl")
    vv = t("vv")
    acc = t("acc")

    def gl(lo, hi, Q, k, phi):
        # batched quadrature: all Ln calls grouped, then all Exp calls, to
        # minimize ACT function-table reloads
        xi, wq = GL_X[Q], GL_W[Q]
        if isinstance(lo, float):
            s.activation(dpl[:], hi, AF.Identity, bias=-lo)
            s.activation(mpl[:], dpl[:], AF.Identity, scale=0.5, bias=lo)
        elif isinstance(hi, float):
            s.activation(dpl[:], lo, AF.Identity, scale=-1.0, bias=hi)
            s.activation(mpl[:], dpl[:], AF.Identity, scale=-0.5, bias=hi)
        else:
            v.tensor_tensor(out=dpl[:], in0=hi, in1=lo, op=OP.subtract)
            v.scalar_tensor_tensor(out=mpl[:], in0=dpl[:], scalar=0.5, in1=lo, op0=OP.mult, op1=OP.add)
        s.activation(hpl[:], dpl[:], AF.Copy, scale=0.5)
        tqs = [t(n) for n in ("Sa", "Sb", "Sab", "pa")[:Q]]
        l2s = [t(n) for n in ("pb", "pab", "lw", "wt")[:Q]]
        for q in range(Q):
            v.scalar_tensor_tensor(out=tqs[q][:], in0=hpl[:], scalar=float(xi[q]), in1=mpl[:], op0=OP.mult, op1=OP.add)
        for q in range(Q):
            s.activation(l2s[q][:], tqs[q][:], AF.Ln, scale=-1.0, bias=1.0)
        for q in range(Q):
            s.activation(tqs[q][:], tqs[q][:], AF.Ln)
        for q in range(Q):
            v.tensor_tensor(out=tqs[q][:], in0=A1[:], in1=tqs[q][:], op=OP.mult)
            v.tensor_tensor(out=vv[:], in0=B1[:], in1=l2s[q][:], op=OP.mult)
            v.tensor_tensor(out=tqs[q][:], in0=tqs[q][:], in1=vv[:], op=OP.add)
        for q in range(Q):
            s.activation(tqs[q][:], tqs[q][:], AF.Exp)
        v.tensor_scalar(out=acc[:], in0=tqs[0][:], scalar1=float(wq[0]), scalar2=None, op0=OP.mult)
        for q in range(1, Q):
            v.scalar_tensor_tensor(out=acc[:], in0=tqs[q][:], scalar=float(wq[q]), in1=acc[:], op0=OP.mult, op1=OP.add)
        v.tensor_tensor(out=acc[:], in0=acc[:], in1=hpl[:], op=OP.mult)
        if phi is not None:
            v.tensor_tensor(out=acc[:], in0=acc[:], in1=phi[:], op=OP.add)
        v.tensor_tensor(out=mav[:, :, k], in0=acc[:], in1=iB[:], op=OP.mult)

    gl(TAU0, x(0), QT, 0, phi0)
    gl(x(0), x(1), QI, 1, None)
    gl(x(1), x(2), QI, 2, None)
    gl(x(2), x(3), QI, 3, None)
    gl(x(3), TAU4, QT, 4, phi4)

    # ---- stores (batch-contiguous; E columns are already batch-ordered) ----
    nc.sync.dma_start(mass_d[csl].rearrange("(p w) d -> p (w d)", p=P), mass[:])
    nc.sync.dma_start(edges_d[csl].rearrange("(p w) d -> p (w d)", p=P), edges[:])


def _register_consts(nc):
    vals = [-1.0, 2.0, TAU0, -TAU0, TAU4, 1.0]
    for n in range(NSER):
        vals.append((1.0 + n) / TAU0)
        vals.append((1.0 + n) / (1.0 - TAU4))
    for v0 in sorted(set(vals)):
        if (F32, v0) in nc.const_aps.aps:
            continue
        tns = nc.alloc_sbuf_tensor(f"cst_f32_{len(nc.const_aps.aps)}", [128, 1], F32)
        nc.gpsimd.memset(tns.ap(), v0)
        nc.const_aps.aps[(F32, v0)] = tns.ap()
    nc.all_engine_barrier()


def build_nc(n_cores=N_CORES, reps=1):
    import os
    bisect = os.environ.get("KBISECT", "full")
    nc = bacc.Bacc("TRN2", target_bir_lowering=False, debug=False,
                   num_devices=n_cores, num_swdge_queues=4,
                   dynamic_dma_scratch_size=32768)
    _register_consts(nc)
    uid_d = nc.dram_tensor("uid", [PER_CORE], I32, kind="ExternalInput").ap()
    iid_d = nc.dram_tensor("iid", [PER_CORE], I32, kind="ExternalInput").ap()
    al_d = nc.dram_tensor("alpha", [PER_CORE], F32, kind="ExternalInput").ap()
    be_d = nc.dram_tensor("beta", [PER_CORE], F32, kind="ExternalInput").ap()
    uw_d = nc.dram_tensor("uid_w", [NROWS, D], F32, kind="ExternalInput").ap()
    iw_d = nc.dram_tensor("iid_w", [NROWS, D], F32, kind="ExternalInput").ap()
    mass_d = nc.dram_tensor("mass", [PER_CORE, D], F32, kind="ExternalOutput").ap()
    edges_d = nc.dram_tensor("edges", [PER_CORE, D], F32, kind="ExternalOutput").ap()
    uwp_d = nc.dram_tensor("uwp", [NROWS, 8], BF16, kind="Internal").ap()
    iwp_d = nc.dram_tensor("iwp", [NROWS, 8], BF16, kind="Internal").ap()
    Lh_d = nc.dram_tensor("Lh", [D, PER_CORE], BF16, kind="Internal").ap()

    ones16 = np.ones((16, 128), dtype=ml_dtypes.bfloat16)
    a8 = np.zeros((128, 8), dtype=ml_dtypes.bfloat16)
    for p in range(128):
        a8[p, p % 8] = 1.0
    cc = np.zeros((128, 2), dtype=np.float32)
    for p in range(128):
        for b in range(2):
            cc[p, b] = 16 * b + p // 8
    i16 = np.eye(16, dtype=ml_dtypes.bfloat16)
    ones16_d = nc.inline_tensor(ones16, "ones16")
    a8_d = nc.inline_tensor(a8, "a8c")
    cc_d = nc.inline_tensor(cc, "ccc")
    i16_d = nc.inline_tensor(i16, "i16c")

    with TileContext(nc) as tc:
        with tc.tile_pool(name="main", bufs=1) as pool, \
             tc.tile_pool(name="psum", bufs=1, space="PSUM") as ppool:
            ones16_t = pool.tile([16, 128], BF16, name="ones16t")
            a8_t = pool.tile([128, 8], BF16, name="a8t")
            cc_t = pool.tile([128, 2], F32, name="cct")
            i16_t = pool.tile([16, 16], BF16, name="i16t")
            nc.sync.dma_start(ones16_t[:], ones16_d.ap())
            nc.sync.dma_start(a8_t[:], a8_d.ap())
            nc.sync.dma_start(cc_t[:], cc_d.ap())
            nc.sync.dma_start(i16_t[:], i16_d.ap())

            cst = (ones16_t, a8_t, cc_t, i16_t)
            gdram = (uid_d, iid_d, uwp_d, iwp_d, Lh_d)
            cdram = (al_d, be_d, Lh_d, mass_d, edges_d)
            for rep in range(reps):
                if bisect != "none":
                    finals = _emit_table_prep(nc, pool, uw_d, uwp_d)
                    finals += _emit_table_prep(nc, pool, iw_d, iwp_d)
                    for fl in finals:
                        nc.gpsimd.memset(fl[0:1, 0:1], 0.0)
                if bisect == "full":
                    for g in range(NG):
                        lsb = _emit_group(nc, pool, ppool, cst, gdram, g)
                        if (g + 1) % GROUPS_PER_CHUNK == 0:
                            _emit_chunk(nc, pool, cdram, lsb,
                                        (g + 1) // GROUPS_PER_CHUNK - 1)
                elif bisect in ("groups", "gather"):
                    for g in range(NG):
                        _emit_group(nc, pool, ppool, cst, gdram, g,
                                    gather_only=(bisect == "gather"))
            if bisect != "full":
                zz = pool.tile([P, W * D], F32, name="zz")
                nc.vector.memset(zz[:], 0.0)
                for c in range(NCHUNK):
                    csl = slice(c * CHUNK, (c + 1) * CHUNK)
                    nc.sync.dma_start(
                        mass_d[csl].rearrange("(p w) d -> p (w d)", p=P), zz[:])
                    nc.sync.dma_start(
                        edges_d[csl].rearrange("(p w) d -> p (w d)", p=P), zz[:])
    nc.compile()
    return nc


_CACHED = {}


def kernel(uid, iid, alpha, beta, uid_w, iid_w):
    uid = np.ascontiguousarray(np.asarray(uid), dtype=np.int32).reshape(-1)
    iid = np.ascontiguousarray(np.asarray(iid), dtype=np.int32).reshape(-1)
    alpha = np.ascontiguousarray(np.asarray(alpha), dtype=np.float32).reshape(-1)
    beta = np.ascontiguousarray(np.asarray(beta), dtype=np.float32).reshape(-1)
    uid_w = np.ascontiguousarray(np.asarray(uid_w), dtype=np.float32)
    iid_w = np.ascontiguousarray(np.asarray(iid_w), dtype=np.float32)
    b = uid.shape[0]
    assert b == B_TOTAL, b

    if "nc1" not in _CACHED:
        _CACHED["nc1"] = build_nc()
    nc = _CACHED["nc1"]

    pc = PER_CORE
    in_maps = []
    for c in range(N_CORES):
        sl = slice(c * pc, (c + 1) * pc)
        in_maps.append({
            "uid": uid[sl], "iid": iid[sl],
            "alpha": alpha[sl], "beta": beta[sl],
            "uid_w": uid_w, "iid_w": iid_w,
        })
    res = bass_utils.run_bass_kernel_spmd(nc, in_maps, core_ids=list(range(N_CORES)))
    mass = np.concatenate([res.results[c]["mass"] for c in range(N_CORES)], axis=0)
    edges = np.concatenate([res.results[c]["edges"] for c in range(N_CORES)], axis=0)
    return mass, edges


def time_exec(inputs, iters=5, reps=1):
    """Time repeated on-device executions with device-resident inputs.

    The wall time includes a large axon dispatch floor (~100ms); use
    reps>1 plus a reps=1 run and difference the minima to get HW time.
    """
    import jax
    from jax.sharding import Mesh, PartitionSpec
    from jax.experimental.shard_map import shard_map
    from concourse import bass2jax

    bass2jax.install_neuronx_cc_hook()
    key = f"nc{reps}"
    if key not in _CACHED:
        _CACHED[key] = build_nc(reps=reps)
    nc = _CACHED[key]
    partition_name = nc.partition_id_tensor.name if nc.partition_id_tensor else None

    in_names, out_names, out_avals = [], [], []
    for alloc in nc.m.functions[0].allocations:
        if not isinstance(alloc, mybir.MemoryLocationSet):
            continue
        if alloc.kind not in ("ExternalInput", "ExternalOutput"):
            continue
        name = alloc.memorylocations[0].name
        if alloc.kind == "ExternalInput":
            if name != partition_name:
                in_names.append(name)
        elif alloc.kind == "ExternalOutput":
            out_names.append(name)
            out_avals.append(jax.core.ShapedArray(tuple(alloc.tensor_shape), mybir.dt.np(alloc.dtype)))
    all_names = in_names + out_names

    bind_names = list(all_names)
    if partition_name is not None:
        bind_names.append(partition_name)

    def _body(*args):
        operands = list(args)
        if partition_name is not None:
            operands.append(bass2jax.partition_id_tensor())
        return tuple(bass2jax._bass_exec_p.bind(
            *operands,
            out_avals=tuple(out_avals),
            in_names=tuple(bind_names),
            out_names=tuple(out_names),
            lowering_input_output_aliases=(),
            sim_require_finite=True,
            sim_require_nnan=True,
            nc=nc,
        ))

    uid = np.ascontiguousarray(np.asarray(inputs["uid"]), dtype=np.int32).reshape(N_CORES, PER_CORE)
    iid = np.ascontiguousarray(np.asarray(inputs["iid"]), dtype=np.int32).reshape(N_CORES, PER_CORE)
    alpha = np.ascontiguousarray(np.asarray(inputs["alpha"]), dtype=np.float32).reshape(N_CORES, PER_CORE)
    beta = np.ascontiguousarray(np.asarray(inputs["beta"]), dtype=np.float32).reshape(N_CORES, PER_CORE)
    uid_w = np.ascontiguousarray(np.asarray(inputs["uid_w"]), dtype=np.float32)
    iid_w = np.ascontiguousarray(np.asarray(inputs["iid_w"]), dtype=np.float32)
    per_name = {
        "uid": uid.reshape(-1), "iid": iid.reshape(-1),
        "alpha": alpha.reshape(-1), "beta": beta.reshape(-1),
        "uid_w": np.concatenate([uid_w] * N_CORES, axis=0),
        "iid_w": np.concatenate([iid_w] * N_CORES, axis=0),
        "mass": np.zeros((N_CORES * PER_CORE, D), np.float32),
        "edges": np.zeros((N_CORES * PER_CORE, D), np.float32),
    }
    devices = jax.devices()[:N_CORES]
    mesh = Mesh(np.asarray(devices), ("core",))
    specs = (PartitionSpec("core"),) * len(all_names)
    out_specs = (PartitionSpec("core"),) * len(out_names)
    fn = jax.jit(shard_map(_body, mesh=mesh, in_specs=specs, out_specs=out_specs, check_rep=False),
                 keep_unused=True)
    import time as _time
    args = [jax.device_put(per_name[n]) for n in all_names]
    outs = fn(*args)
    jax.block_until_ready(outs)
    times = []
    for _ in range(iters):
        t0 = _time.time()
        outs = fn(*args)
        jax.block_until_ready(outs)
        times.append(_time.time() - t0)
    return times



# revision 2
# speedup vs baseline: 15.8733x; 15.8733x over previous
"""Trainium2 Bass kernel for nn_BetaBinsMassAdaptive (v2: dma_gather windows).

Computes, for each batch element:
  logits = uid_w[uid] + iid_w[iid]            (gather from two [1M,5] tables)
  bins   = softmax(logits); edges = cumsum(bins)
  cdf    = betainc(alpha, beta, edges[:4]); mass = diff([0, cdf, 1])
Returns (mass, edges), both [B,5] float32.

Gather strategy (replaces 8192 per-core indirect_dma_start calls, which are
descriptor-generation bound at ~17us each):
  - tables are padded on-device to [1M, 8] bf16 (16B rows) in HBM scratch
  - one InstDMAGatherAnt per 4096 elements fetches 512B windows of 32 rows
    (idx = uid>>5 fits the gather's int16 index limit) in xbar-transpose
    layout: window u16 position w -> partition w%128, block w//128, col=elem
  - the exact row (lo = uid&31) is selected on-chip: a one-hot mask built by
    comparing a broadcast of lo against a per-partition constant, then a
    [128x8] constant matmul folds the 16 candidate partition-positions down
    to 8 output rows; uid/iid selects accumulate in PSUM -> logits [5, G]
  - logits staged to HBM [5, N] and re-read in [128, W] element-major tiles
    for the betainc pipeline (same validated GL4+series math as v1)

betainc strategy (validated to ~8e-6 max abs err vs float64):
  mass_0 = I_tau0(a,b)        + GL4 integral over [tau0, x0]
  mass_k = GL4 integral over [x_{k-1}, x_k]          (k=1,2,3)
  mass_4 = I_{1-tau4}(b,a)    + GL4 integral over [x3, tau4]
  I_tau(p,q) at FIXED tau via the 2F1 series (N=8 terms), 1/B via Stirling.

Sharding: batch split evenly across 8 NeuronCores; tables replicated.
"""
import sys

sys.path.insert(0, "/opt/trn_rl_repo")

import os as _os

import numpy as np
import ml_dtypes

_NOINC = _os.environ.get("KNOINC", "") == "1"

import concourse.bass as bass
import concourse.bacc as bacc
import concourse.mybir as mybir
from concourse.tile import TileContext
from concourse import bass_utils

F32 = mybir.dt.float32
BF16 = mybir.dt.bfloat16
I32 = mybir.dt.int32
I16 = mybir.dt.int16
AF = mybir.ActivationFunctionType
OP = mybir.AluOpType

P = 128
N_CORES = 8
B_TOTAL = 4_194_304
PER_CORE = B_TOTAL // N_CORES      # 524288
NROWS = 1_000_000
D = 5

# gather phase
G = 4096                           # elements per gather group
GW = G // 16                       # idx cols (wrap-16)
NG = PER_CORE // G                 # 128 groups
PIECE = 512                        # PE/mask piece (PSUM bank width)
NPIECE = G // PIECE                # 8
NSPLIT = 1                         # gathers per table (4096-idx gathers: ~8.0ns/desc vs 9.9 at 2048)
GS = G // NSPLIT                   # idxs per gather
NWIN = NROWS // 32                 # 31250 windows of 256 bf16

# betainc phase
CHUNK = 32768
W = CHUNK // P                     # 256
NCHUNK = PER_CORE // CHUNK         # 16
GROUPS_PER_CHUNK = CHUNK // G      # 8

TAU0 = 0.107
TAU4 = 1.0 - TAU0
NSER = 6
QT = 4
QI = 3

GL_X = {
    3: [-0.7745966692414834, 0.0, 0.7745966692414834],
    4: [-0.8611363115940526, -0.33998104358485626, 0.33998104358485626, 0.8611363115940526],
}
GL_W = {
    3: [0.5555555555555556, 0.8888888888888888, 0.5555555555555556],
    4: [0.34785484513745385, 0.6521451548625461, 0.6521451548625461, 0.34785484513745385],
}

HALF_LN_2PI = 0.9189385332046727

# table prep tiling
PREP_R = 16384                     # rows per prep tile (128 x 128 x 5)
PREP_NT = NROWS // PREP_R          # 61 full tiles
PREP_TAIL = NROWS - PREP_NT * PREP_R   # 576 = 64 x 9
PREP_STORES = PREP_NT + 1          # 62 stores per table


def _emit_table_prep(nc, pool, w_d, wp_d):
    """Pad/cast one [1M,5] f32 table to [1M,8] bf16 in HBM scratch.

    Returns the final dst tiles; a later gpsimd write to their corners (WAR
    on the store DMAs) gates the gathers behind prep completion.
    """
    v = nc.vector
    finals = []
    for t in range(PREP_NT):
        src = pool.tile([P, 128 * D], F32, name="psrc", bufs=1)
        dst = pool.tile([P, 128 * 8], BF16, name="pdst", bufs=2)
        if t >= PREP_NT - 2:
            finals.append(dst)
        sl = slice(t * PREP_R, (t + 1) * PREP_R)
        nc.sync.dma_start(src[:], w_d[sl].rearrange("(p r) d -> p (r d)", p=P))
        dstv = dst[:].rearrange("p (r d) -> p r d", d=8)
        v.memset(dstv[:, :, D:8], 0.0)
        v.tensor_copy(out=dstv[:, :, 0:D],
                      in_=src[:].rearrange("p (r d) -> p r d", d=D))
        nc.sync.dma_start(
            wp_d[sl].rearrange("(p r) d -> p (r d)", p=P), dst[:])
    # tail: 576 rows = 64 partitions x 9 rows
    src = pool.tile([64, 9 * D], F32, name="psrct", bufs=1)
    dst = pool.tile([64, 9 * 8], BF16, name="pdstt", bufs=1)
    sl = slice(PREP_NT * PREP_R, NROWS)
    nc.sync.dma_start(src[:], w_d[sl].rearrange("(p r) d -> p (r d)", p=64))
    dstv = dst[:].rearrange("p (r d) -> p r d", d=8)
    v.memset(dstv[:, :, D:8], 0.0)
    v.tensor_copy(out=dstv[:, :, 0:D],
                  in_=src[:].rearrange("p (r d) -> p r d", d=D))
    nc.sync.dma_start(
        wp_d[sl].rearrange("(p r) d -> p (r d)", p=64), dst[:])
    finals.append(dst)
    return finals


def _emit_group(nc, pool, ppool, cst, dram, g, gather_only=False):
    """Gather + select logits for elements [g*G, (g+1)*G) -> Lh columns.

    Returns the L_sb tile whose store carries this group's Lh write."""
    v = nc.vector
    s = nc.scalar
    uid_d, iid_d, uwp_d, iwp_d, Lh_d = dram
    ones16_t, a8_t, cc_t, i16_t = cst

    sl = slice(g * G, (g + 1) * G)
    win_v = {}
    for t, (idx_d, wp_d) in enumerate(((uid_d, uwp_d), (iid_d, iwp_d))):
        nm = "ui"[t]
        # All gathers on queue 0: the xbar-transpose drain path corrupts
        # data when two queues' transpose gathers run concurrently.
        qn = 0
        pb = 0
        # wrap-16 permutes only within each 256-element block: gather pos
        # i = blk*256 + r reads batch element blk*256 + (r%16)*16 + r//16,
        # so idx32[p, blk*16+w] = uid[g*G + blk*256 + p*16 + w].
        idx32 = pool.tile([16, GW], I32, name=f"idx32{nm}", bufs=2)
        nc.sync.dma_start(
            idx32[:].rearrange("p (blk w) -> p blk w", w=16),
            idx_d[sl].rearrange("(blk p w) -> p blk w", p=16, w=16))
        hi32 = pool.tile([16, GW], I32, name=f"hi32{nm}", bufs=1)
        v.tensor_scalar(out=hi32[:], in0=idx32[:], scalar1=5, scalar2=None,
                        op0=OP.arith_shift_right)
        idx16 = pool.tile([32, GW], I16, name=f"idx16{nm}", bufs=2)
        v.tensor_copy(out=idx16[pb:pb + 16, :], in_=hi32[:])
        nc.sync.dma_start(idx16[pb + 16:pb + 32, :], idx16[pb:pb + 16, :])
        lo32 = pool.tile([16, GW], I32, name=f"lo32{nm}", bufs=1)
        v.tensor_scalar(out=lo32[:], in0=idx32[:], scalar1=31, scalar2=None,
                        op0=OP.bitwise_and)
        lo_bf = pool.tile([16, GW], BF16, name=f"lobf{nm}", bufs=1)
        v.tensor_copy(out=lo_bf[:], in_=lo32[:])
        spread = pool.tile([16, GW * 16], BF16, name=f"spread{nm}", bufs=1)
        nc.gpsimd.affine_select(
            out=spread[:].rearrange("p (w j) -> p w j", j=16),
            in_=lo_bf[:].rearrange("p (w o) -> p w o", o=1).broadcast_to([16, GW, 16]),
            pattern=[[0, GW], [1, 16]],
            compare_op=OP.is_equal,
            fill=0.0,
            base=0,
            channel_multiplier=-1)
        wins = []
        for h in range(NSPLIT):
            win = pool.tile([P, 2 * GS], BF16, name=f"win{nm}{h}", bufs=2)
            # single_packet=False: coalescing would exceed the 64-desc
            # packet limit (tx descs/engine > 64) and faults the device
            nc.gpsimd.dma_gather(
                win[:].rearrange("p (b e) -> p b e", b=2),
                wp_d[:].rearrange("(w k) d -> w (k d)", k=32),
                idx16[:, h * (GS // 16):(h + 1) * (GS // 16)],
                GS, GS, 256, elem_step=256, transpose=True,
                single_packet=False, queue_num=qn)
            wins.append(win)
        win_v[nm] = (wins, spread)

    if gather_only:
        # consume win/spread trivially so Tile rotation stays sane
        zsink = pool.tile([P, 8], BF16, name="zsink", bufs=1)
        for nm in ("u", "i"):
            wins, spread = win_v[nm]
            for win in wins:
                nc.vector.tensor_copy(out=zsink[:], in_=win[:, 0:8])
        return None

    L_sb = pool.tile([8, G], BF16, name="Lsb", bufs=2)
    for sp in range(NPIECE):
        ps = slice(sp * PIECE, (sp + 1) * PIECE)
        hsp = sp // (NPIECE // NSPLIT)
        psl = slice((sp % (NPIECE // NSPLIT)) * PIECE,
                    (sp % (NPIECE // NSPLIT) + 1) * PIECE)
        gms = []
        for nm in ("u", "i"):
            wins, spread = win_v[nm]
            winv = wins[hsp][:].rearrange("p (b e) -> p b e", b=2)
            lo_ps = ppool.tile([P, PIECE], F32, name=f"lops{nm}", bufs=1,
                               space="PSUM")
            nc.tensor.matmul(out=lo_ps[:], lhsT=ones16_t[:],
                             rhs=spread[:, ps], start=True, stop=True)
            gm0 = pool.tile([P, PIECE], BF16, name=f"gm0{nm}", bufs=1)
            gm1 = pool.tile([P, PIECE], BF16, name=f"gm1{nm}", bufs=1)
            v.scalar_tensor_tensor(out=gm0[:], in0=lo_ps[:], scalar=cc_t[:, 0:1],
                                   in1=winv[:, 0, psl], op0=OP.is_equal, op1=OP.mult)
            v.scalar_tensor_tensor(out=gm1[:], in0=lo_ps[:], scalar=cc_t[:, 1:2],
                                   in1=winv[:, 1, psl], op0=OP.is_equal, op1=OP.mult)
            gms += [gm0, gm1]
        L_ps = ppool.tile([8, PIECE], F32, name="Lps", bufs=2, space="PSUM")
        for k, gm in enumerate(gms):
            nc.tensor.matmul(out=L_ps[:], lhsT=a8_t[:], rhs=gm[:],
                             start=(k == 0), stop=(k == len(gms) - 1))
        s.activation(L_sb[:, ps], L_ps[:], AF.Copy)

    nc.sync.dma_start(Lh_d[:, sl], L_sb[0:5, :])
    return L_sb


def _emit_chunk(nc, pool, dram, last_lsb, c):
    """betainc pipeline for chunk c (CHUNK elements), reading staged logits.

    last_lsb: L_sb tile of the chunk's final group. A corner write to it
    (WAR) waits for that group's Lh store; per-ring FIFO completion means
    all 8 group stores are then done, and the WAW chain onto Lt delays the
    logits load until the corner write completes."""
    v = nc.vector
    s = nc.scalar
    al_d, be_d, Lh_d, mass_d, edges_d = dram

    def t(name, width=W, dtype=F32, bufs=1):
        return pool.tile([P, width], dtype, name=name, tag=name, bufs=bufs)

    # ---- loads ----
    Lt = t("Lt", W * D, BF16, bufs=2)
    nc.sync.dma_start(last_lsb[0:1, 0:8], Lt[0:1, 0:8])      # t1: WAR on L-stores
    nc.sync.dma_start(Lt[0:1, 0:8], last_lsb[0:1, 0:8])      # t2: RAW t1 -> WAW gates load
    nc.sync.dma_start(
        Lt[:].rearrange("p (d j) -> p d j", d=D),
        Lh_d[:, c * CHUNK:(c + 1) * CHUNK].rearrange("d (q j) -> q d j", q=P))

    al = t("al", bufs=2)
    be = t("be", bufs=2)
    csl = slice(c * CHUNK, (c + 1) * CHUNK)
    nc.sync.dma_start(al[:], al_d[csl].rearrange("(p w) -> p w", p=P))
    nc.sync.dma_start(be[:], be_d[csl].rearrange("(p w) -> p w", p=P))

    # ---- softmax -> edges ----
    # E column u holds batch element c*CHUNK + q*256 + u, whose logits sit at
    # gather position j = (u%16)*16 + u//16 within the partition's Lt row.
    E = t("E", W * D)
    v.tensor_copy(
        out=E[:].rearrange("p (j16 jh d) -> p j16 jh d", jh=16, j16=16, d=D),
        in_=Lt[:].rearrange("p (d jh j16) -> p j16 jh d", d=D, jh=16, j16=16))
    s.activation(E[:], E[:], AF.Exp)
    Ev = E[:].rearrange("p (w d) -> p w d", d=D)

    cum = t("cum", W * D)
    cumv = cum[:].rearrange("p (w d) -> p w d", d=D)
    s.activation(cumv[:, :, 0], Ev[:, :, 0], AF.Copy)
    for j in range(1, D):
        v.tensor_tensor(out=cumv[:, :, j], in0=cumv[:, :, j - 1], in1=Ev[:, :, j], op=OP.add)

    invS = t("invS")
    scr = t("scr")
    v.reciprocal_approx_accurate(out=invS[:], in_=cumv[:, :, D - 1], scratch=scr[:])

    edges = t("edges", W * D, bufs=2)
    edv = edges[:].rearrange("p (w d) -> p w d", d=D)
    invSb = invS[:].rearrange("p (w o) -> p w o", o=1).broadcast_to([P, W, D])
    v.tensor_tensor(out=edv[:], in0=cumv[:], in1=invSb, op=OP.mult)

    def x(k):
        return edv[:, :, k]

    # ---- per-element prep ----
    ab = t("ab")
    v.tensor_tensor(out=ab[:], in0=al[:], in1=be[:], op=OP.add)
    A1 = t("A1")
    B1 = t("B1")
    s.activation(A1[:], al[:], AF.Identity, bias=-1.0)
    s.activation(B1[:], be[:], AF.Identity, bias=-1.0)
    inv_a = t("inv_a")
    inv_b = t("inv_b")
    v.reciprocal_approx_fast(out=inv_a[:], in_=al[:])
    v.reciprocal_approx_fast(out=inv_b[:], in_=be[:])

    # ---- lnB(a,b) via Stirling shift-2 ----
    def stirling(z, out):
        lw = t("lw")
        wt = t("wt")
        iw = t("iw")
        t1 = t("t1")
        u = t("u2")
        s.activation(lw[:], z[:], AF.Ln, bias=2.0)
        s.activation(wt[:], z[:], AF.Identity, bias=2.0)
        v.reciprocal_approx_fast(out=iw[:], in_=wt[:])
        v.scalar_tensor_tensor(out=t1[:], in0=z[:], scalar=1.5, in1=lw[:], op0=OP.add, op1=OP.mult)
        v.tensor_tensor(out=u[:], in0=iw[:], in1=iw[:], op=OP.mult)
        v.tensor_scalar(out=u[:], in0=u[:], scalar1=-1.0 / 360.0, scalar2=1.0 / 12.0, op0=OP.mult, op1=OP.add)
        v.tensor_tensor(out=u[:], in0=iw[:], in1=u[:], op=OP.mult)
        v.tensor_tensor(out=t1[:], in0=t1[:], in1=wt[:], op=OP.subtract)
        v.scalar_tensor_tensor(out=out[:], in0=u[:], scalar=HALF_LN_2PI, in1=t1[:], op0=OP.add, op1=OP.add)

    Sa = t("Sa")
    Sb = t("Sb")
    Sab = t("Sab")
    stirling(al, Sa)
    stirling(be, Sb)
    stirling(ab, Sab)
    lnB = t("lnB")
    v.tensor_tensor(out=lnB[:], in0=Sa[:], in1=Sb[:], op=OP.add)
    v.tensor_tensor(out=lnB[:], in0=lnB[:], in1=Sab[:], op=OP.subtract)
    pa = t("pa")
    pb = t("pb")
    pab = t("pab")
    v.scalar_tensor_tensor(out=pa[:], in0=al[:], scalar=1.0, in1=al[:], op0=OP.add, op1=OP.mult)
    v.scalar_tensor_tensor(out=pb[:], in0=be[:], scalar=1.0, in1=be[:], op0=OP.add, op1=OP.mult)
    v.scalar_tensor_tensor(out=pab[:], in0=ab[:], scalar=1.0, in1=ab[:], op0=OP.add, op1=OP.mult)
    s.activation(pa[:], pa[:], AF.Ln)
    s.activation(pb[:], pb[:], AF.Ln)
    s.activation(pab[:], pab[:], AF.Ln)
    v.tensor_tensor(out=lnB[:], in0=lnB[:], in1=pab[:], op=OP.add)
    v.tensor_tensor(out=pa[:], in0=pa[:], in1=pb[:], op=OP.add)
    v.tensor_tensor(out=lnB[:], in0=lnB[:], in1=pa[:], op=OP.subtract)
    iB = t("iB")
    s.activation(iB[:], lnB[:], AF.Exp, scale=-1.0)

    # ---- tail series: phi = front * 2F1(1, p+q; p+1; tau) ----
    def tail_series(p_pl, q_pl, invp, tau, out):
        lt = float(np.log(tau))
        l1t = float(np.log1p(-tau))
        e1 = t("e1")
        tt = t("tt")
        un = t("un")
        iu = t("iu")
        v.tensor_scalar(out=e1[:], in0=p_pl[:], scalar1=lt, scalar2=None, op0=OP.mult)
        v.scalar_tensor_tensor(out=e1[:], in0=q_pl[:], scalar=l1t, in1=e1[:], op0=OP.mult, op1=OP.add)
        s.activation(e1[:], e1[:], AF.Exp)
        v.tensor_tensor(out=tt[:], in0=e1[:], in1=invp[:], op=OP.mult)
        s.activation(out[:], tt[:], AF.Copy)
        for n in range(NSER):
            s.activation(un[:], p_pl[:], AF.Identity, scale=1.0 / tau, bias=(1.0 + n) / tau)
            v.reciprocal_approx_fast(out=iu[:], in_=un[:])
            v.tensor_tensor(out=tt[:], in0=tt[:], in1=iu[:], op=OP.mult)
            v.scalar_tensor_tensor(out=tt[:], in0=ab[:], scalar=float(n), in1=tt[:], op0=OP.add, op1=OP.mult)
            v.tensor_tensor(out=out[:], in0=out[:], in1=tt[:], op=OP.add)

    phi0 = t("phi0")
    phi4 = t("phi4")
    tail_series(al, be, inv_a, TAU0, phi0)
    tail_series(be, al, inv_b, 1.0 - TAU4, phi4)

    # ---- GL integrals ----
    mass = t("mass", W * D, bufs=2)
    mav = mass[:].rearrange("p (w d) -> p w d", d=D)

    dpl = t("dpl")
    hpl = t("hpl")
    mpl = t("mpl")
    vv = t("vv")
    acc = t("acc")

    def gl(lo, hi, Q, k, phi):
        # batched quadrature: all Ln calls grouped, then all Exp calls, to
        # minimize ACT function-table reloads
        xi, wq = GL_X[Q], GL_W[Q]
        if isinstance(lo, float):
            s.activation(dpl[:], hi, AF.Identity, bias=-lo)
            s.activation(mpl[:], dpl[:], AF.Identity, scale=0.5, bias=lo)
        elif isinstance(hi, float):
            s.activation(dpl[:], lo, AF.Identity, scale=-1.0, bias=hi)
            s.activation(mpl[:], dpl[:], AF.Identity, scale=-0.5, bias=hi)
        else:
            v.tensor_tensor(out=dpl[:], in0=hi, in1=lo, op=OP.subtract)
            v.scalar_tensor_tensor(out=mpl[:], in0=dpl[:], scalar=0.5, in1=lo, op0=OP.mult, op1=OP.add)
        s.activation(hpl[:], dpl[:], AF.Copy, scale=0.5)
        tqs = [t(n) for n in ("Sa", "Sb", "Sab", "pa")[:Q]]
        l2s = [t(n) for n in ("pb", "pab", "lw", "wt")[:Q]]
        for q in range(Q):
            v.scalar_tensor_tensor(out=tqs[q][:], in0=hpl[:], scalar=float(xi[q]), in1=mpl[:], op0=OP.mult, op1=OP.add)
        for q in range(Q):
            s.activation(l2s[q][:], tqs[q][:], AF.Ln, scale=-1.0, bias=1.0)
        for q in range(Q):
            s.activation(tqs[q][:], tqs[q][:], AF.Ln)
        for q in range(Q):
            v.tensor_tensor(out=tqs[q][:], in0=A1[:], in1=tqs[q][:], op=OP.mult)
            v.tensor_tensor(out=vv[:], in0=B1[:], in1=l2s[q][:], op=OP.mult)
            v.tensor_tensor(out=tqs[q][:], in0=tqs[q][:], in1=vv[:], op=OP.add)
        for q in range(Q):
            s.activation(tqs[q][:], tqs[q][:], AF.Exp)
        v.tensor_scalar(out=acc[:], in0=tqs[0][:], scalar1=float(wq[0]), scalar2=None, op0=OP.mult)
        for q in range(1, Q):
            v.scalar_tensor_tensor(out=acc[:], in0=tqs[q][:], scalar=float(wq[q]), in1=acc[:], op0=OP.mult, op1=OP.add)
        v.tensor_tensor(out=acc[:], in0=acc[:], in1=hpl[:], op=OP.mult)
        if phi is not None:
            v.tensor_tensor(out=acc[:], in0=acc[:], in1=phi[:], op=OP.add)
        v.tensor_tensor(out=mav[:, :, k], in0=acc[:], in1=iB[:], op=OP.mult)

    gl(TAU0, x(0), QT, 0, phi0)
    gl(x(0), x(1), QI, 1, None)
    gl(x(1), x(2), QI, 2, None)
    gl(x(2), x(3), QI, 3, None)
    gl(x(3), TAU4, QT, 4, phi4)

    # ---- stores (batch-contiguous; E columns are already batch-ordered) ----
    nc.sync.dma_start(mass_d[csl].rearrange("(p w) d -> p (w d)", p=P), mass[:])
    nc.sync.dma_start(edges_d[csl].rearrange("(p w) d -> p (w d)", p=P), edges[:])


def _register_consts(nc):
    vals = [-1.0, 2.0, TAU0, -TAU0, TAU4, 1.0]
    for n in range(NSER):
        vals.append((1.0 + n) / TAU0)
        vals.append((1.0 + n) / (1.0 - TAU4))
    for v0 in sorted(set(vals)):
        if (F32, v0) in nc.const_aps.aps:
            continue
        tns = nc.alloc_sbuf_tensor(f"cst_f32_{len(nc.const_aps.aps)}", [128, 1], F32)
        nc.gpsimd.memset(tns.ap(), v0)
        nc.const_aps.aps[(F32, v0)] = tns.ap()
    nc.all_engine_barrier()


def build_nc(n_cores=N_CORES, reps=1):
    import os
    bisect = os.environ.get("KBISECT", "full")
    nc = bacc.Bacc("TRN2", target_bir_lowering=False, debug=False,
                   num_devices=n_cores, num_swdge_queues=4,
                   dynamic_dma_scratch_size=32768)
    _register_consts(nc)
    uid_d = nc.dram_tensor("uid", [PER_CORE], I32, kind="ExternalInput").ap()
    iid_d = nc.dram_tensor("iid", [PER_CORE], I32, kind="ExternalInput").ap()
    al_d = nc.dram_tensor("alpha", [PER_CORE], F32, kind="ExternalInput").ap()
    be_d = nc.dram_tensor("beta", [PER_CORE], F32, kind="ExternalInput").ap()
    uw_d = nc.dram_tensor("uid_w", [NROWS, D], F32, kind="ExternalInput").ap()
    iw_d = nc.dram_tensor("iid_w", [NROWS, D], F32, kind="ExternalInput").ap()
    mass_d = nc.dram_tensor("mass", [PER_CORE, D], F32, kind="ExternalOutput").ap()
    edges_d = nc.dram_tensor("edges", [PER_CORE, D], F32, kind="ExternalOutput").ap()
    uwp_d = nc.dram_tensor("uwp", [NROWS, 8], BF16, kind="Internal").ap()
    iwp_d = nc.dram_tensor("iwp", [NROWS, 8], BF16, kind="Internal").ap()
    Lh_d = nc.dram_tensor("Lh", [D, PER_CORE], BF16, kind="Internal").ap()

    ones16 = np.ones((16, 128), dtype=ml_dtypes.bfloat16)
    a8 = np.zeros((128, 8), dtype=ml_dtypes.bfloat16)
    for p in range(128):
        a8[p, p % 8] = 1.0
    cc = np.zeros((128, 2), dtype=np.float32)
    for p in range(128):
        for b in range(2):
            cc[p, b] = 16 * b + p // 8
    i16 = np.eye(16, dtype=ml_dtypes.bfloat16)
    ones16_d = nc.inline_tensor(ones16, "ones16")
    a8_d = nc.inline_tensor(a8, "a8c")
    cc_d = nc.inline_tensor(cc, "ccc")
    i16_d = nc.inline_tensor(i16, "i16c")

    with TileContext(nc) as tc:
        with tc.tile_pool(name="main", bufs=1) as pool, \
             tc.tile_pool(name="psum", bufs=1, space="PSUM") as ppool:
            ones16_t = pool.tile([16, 128], BF16, name="ones16t")
            a8_t = pool.tile([128, 8], BF16, name="a8t")
            cc_t = pool.tile([128, 2], F32, name="cct")
            i16_t = pool.tile([16, 16], BF16, name="i16t")
            nc.sync.dma_start(ones16_t[:], ones16_d.ap())
            nc.sync.dma_start(a8_t[:], a8_d.ap())
            nc.sync.dma_start(cc_t[:], cc_d.ap())
            nc.sync.dma_start(i16_t[:], i16_d.ap())

            cst = (ones16_t, a8_t, cc_t, i16_t)
            gdram = (uid_d, iid_d, uwp_d, iwp_d, Lh_d)
            cdram = (al_d, be_d, Lh_d, mass_d, edges_d)
            for rep in range(reps):
                if bisect != "none":
                    finals = _emit_table_prep(nc, pool, uw_d, uwp_d)
                    finals += _emit_table_prep(nc, pool, iw_d, iwp_d)
                    for fl in finals:
                        nc.gpsimd.memset(fl[0:1, 0:1], 0.0)
                if bisect == "full":
                    for g in range(NG):
                        lsb = _emit_group(nc, pool, ppool, cst, gdram, g)
                        if (g + 1) % GROUPS_PER_CHUNK == 0:
                            _emit_chunk(nc, pool, cdram, lsb,
                                        (g + 1) // GROUPS_PER_CHUNK - 1)
                elif bisect in ("groups", "gather"):
                    for g in range(NG):
                        _emit_group(nc, pool, ppool, cst, gdram, g,
                                    gather_only=(bisect == "gather"))
            if bisect != "full":
                zz = pool.tile([P, W * D], F32, name="zz")
                nc.vector.memset(zz[:], 0.0)
                for c in range(NCHUNK):
                    csl = slice(c * CHUNK, (c + 1) * CHUNK)
                    nc.sync.dma_start(
                        mass_d[csl].rearrange("(p w) d -> p (w d)", p=P), zz[:])
                    nc.sync.dma_start(
                        edges_d[csl].rearrange("(p w) d -> p (w d)", p=P), zz[:])
    nc.compile()
    return nc


_CACHED = {}


def kernel(uid, iid, alpha, beta, uid_w, iid_w):
    uid = np.ascontiguousarray(np.asarray(uid), dtype=np.int32).reshape(-1)
    iid = np.ascontiguousarray(np.asarray(iid), dtype=np.int32).reshape(-1)
    alpha = np.ascontiguousarray(np.asarray(alpha), dtype=np.float32).reshape(-1)
    beta = np.ascontiguousarray(np.asarray(beta), dtype=np.float32).reshape(-1)
    uid_w = np.ascontiguousarray(np.asarray(uid_w), dtype=np.float32)
    iid_w = np.ascontiguousarray(np.asarray(iid_w), dtype=np.float32)
    b = uid.shape[0]
    assert b == B_TOTAL, b

    if "nc1" not in _CACHED:
        _CACHED["nc1"] = build_nc()
    nc = _CACHED["nc1"]

    pc = PER_CORE
    in_maps = []
    for c in range(N_CORES):
        sl = slice(c * pc, (c + 1) * pc)
        in_maps.append({
            "uid": uid[sl], "iid": iid[sl],
            "alpha": alpha[sl], "beta": beta[sl],
            "uid_w": uid_w, "iid_w": iid_w,
        })
    res = bass_utils.run_bass_kernel_spmd(nc, in_maps, core_ids=list(range(N_CORES)))
    mass = np.concatenate([res.results[c]["mass"] for c in range(N_CORES)], axis=0)
    edges = np.concatenate([res.results[c]["edges"] for c in range(N_CORES)], axis=0)
    return mass, edges


def time_exec(inputs, iters=5, reps=1):
    """Time repeated on-device executions with device-resident inputs.

    The wall time includes a large axon dispatch floor (~100ms); use
    reps>1 plus a reps=1 run and difference the minima to get HW time.
    """
    import jax
    from jax.sharding import Mesh, PartitionSpec
    from jax.experimental.shard_map import shard_map
    from concourse import bass2jax

    bass2jax.install_neuronx_cc_hook()
    key = f"nc{reps}"
    if key not in _CACHED:
        _CACHED[key] = build_nc(reps=reps)
    nc = _CACHED[key]
    partition_name = nc.partition_id_tensor.name if nc.partition_id_tensor else None

    in_names, out_names, out_avals = [], [], []
    for alloc in nc.m.functions[0].allocations:
        if not isinstance(alloc, mybir.MemoryLocationSet):
            continue
        if alloc.kind not in ("ExternalInput", "ExternalOutput"):
            continue
        name = alloc.memorylocations[0].name
        if alloc.kind == "ExternalInput":
            if name != partition_name:
                in_names.append(name)
        elif alloc.kind == "ExternalOutput":
            out_names.append(name)
            out_avals.append(jax.core.ShapedArray(tuple(alloc.tensor_shape), mybir.dt.np(alloc.dtype)))
    all_names = in_names + out_names

    bind_names = list(all_names)
    if partition_name is not None:
        bind_names.append(partition_name)

    def _body(*args):
        operands = list(args)
        if partition_name is not None:
            operands.append(bass2jax.partition_id_tensor())
        return tuple(bass2jax._bass_exec_p.bind(
            *operands,
            out_avals=tuple(out_avals),
            in_names=tuple(bind_names),
            out_names=tuple(out_names),
            lowering_input_output_aliases=(),
            sim_require_finite=True,
            sim_require_nnan=True,
            nc=nc,
        ))

    uid = np.ascontiguousarray(np.asarray(inputs["uid"]), dtype=np.int32).reshape(N_CORES, PER_CORE)
    iid = np.ascontiguousarray(np.asarray(inputs["iid"]), dtype=np.int32).reshape(N_CORES, PER_CORE)
    alpha = np.ascontiguousarray(np.asarray(inputs["alpha"]), dtype=np.float32).reshape(N_CORES, PER_CORE)
    beta = np.ascontiguousarray(np.asarray(inputs["beta"]), dtype=np.float32).reshape(N_CORES, PER_CORE)
    uid_w = np.ascontiguousarray(np.asarray(inputs["uid_w"]), dtype=np.float32)
    iid_w = np.ascontiguousarray(np.asarray(inputs["iid_w"]), dtype=np.float32)
    per_name = {
        "uid": uid.reshape(-1), "iid": iid.reshape(-1),
        "alpha": alpha.reshape(-1), "beta": beta.reshape(-1),
        "uid_w": np.concatenate([uid_w] * N_CORES, axis=0),
        "iid_w": np.concatenate([iid_w] * N_CORES, axis=0),
        "mass": np.zeros((N_CORES * PER_CORE, D), np.float32),
        "edges": np.zeros((N_CORES * PER_CORE, D), np.float32),
    }
    devices = jax.devices()[:N_CORES]
    mesh = Mesh(np.asarray(devices), ("core",))
    specs = (PartitionSpec("core"),) * len(all_names)
    out_specs = (PartitionSpec("core"),) * len(out_names)
    fn = jax.jit(shard_map(_body, mesh=mesh, in_specs=specs, out_specs=out_specs, check_rep=False),
                 keep_unused=True)
    import time as _time
    args = [jax.device_put(per_name[n]) for n in all_names]
    outs = fn(*args)
    jax.block_until_ready(outs)
    times = []
    for _ in range(iters):
        t0 = _time.time()
        outs = fn(*args)
        jax.block_until_ready(outs)
        times.append(_time.time() - t0)
    return times

